# revision 7
# baseline (speedup 1.0000x reference)
"""ArcFace loss kernel for 8 TRN2 NeuronCores (vocab/tensor-parallel).

reference:
    xn = normalize(x)               # [B, D]
    wn = normalize(weight)          # [C, D]
    logits = 64 * xn @ wn.T         # [B, C]
    loss = mean(CE(logits, label))

Strategy: shard classes C=100000 over 8 cores (12500 each, zero-padded to
12544 = 24*512 + 256). Host prepares normalized fp8(e4m3) operands scaled
by G=8 (so device cosines are 64*cos), pre-packed in the exact SBUF tile
layout so every weight-group DMA is 8KB-contiguous per partition. Each core
computes its logit shard with TensorE fp8 DoubleRow matmuls (K=256/op) into
fp32 PSUM.

The sum-of-exp over each PSUM tile is split across two engines so the PE
stream (8 x 216ns matmuls = ~1.73us/tile) is the only bottleneck:
  - ScalarE: one fused exp(l - SHIFT) activation over a dedicated 3-bank
    PSUM tile (1536 cols) with row-accumulate (~1.54us + 182ns drain).
  - VectorE: Schraudolph exp on a dedicated 1-bank PSUM tile (512 cols):
    i32 = rint(A*l + Beff) via one tensor_scalar (fp32 mul-add, HW
    verified round-to-nearest), whose int32 bits ARE the fp32 exp image;
    a tensor_reduce over the bitcast sums it (~1.4us total). The
    mean-centering constant absorbs the (1+f)2^-f mantissa sawtooth
    (E[g]=1.040684); residual loss error ~1e-4 relative.
  The two consumers read DISJOINT PSUM tiles (separate pools): Tile's
  access tracker serializes cross-engine readers of one tile, which
  otherwise stalls the PE every other tile and trips HAM re-throttling.

DMA: every HWDGE trigger costs ~650ns of sequencer time, serially, so
transfers are packed to minimize trigger count ahead of the critical
path: x and w0 ride one dram tensor (wx0, k-halved so the first matmuls
start ~1.2us earlier), w5 and the tail share another. All on the sync
ring in exact consumption order; one ring already sprays all 16 SDMA
engines at ~420GB/s aggregate.

Tail tiles (256 cols) go entirely to VectorE (ScalarE runs at zero slack);
the 44 zero-pad columns contribute a bit-deterministic Schraudolph image
of exp(-SHIFT) that the host subtracts exactly. Each core returns per-row
partial sums [128, 4]; the host gathers the 8 cores and finishes
loss = mean(log Z + SHIFT - 64*cos_label) with host-exact label cosines.
"""

import math
import numpy as np

import concourse.mybir as mybir
import concourse.tile as tile
from concourse import bacc
from concourse.bass_utils import run_bass_kernel_spmd

# Problem constants (hardcoded per harness contract).
B = 512
D = 512
C = 100000
S = 64.0
SHIFT = 20.0  # logsumexp shift; keeps Z ~1e-2
EPS = 1e-12
G = 8.0      # fp8 pre-scale on both operands: device cos' = G^2 * cos
NCORES = 8
CS = C // NCORES        # true classes per core = 12500
CHUNK = 512             # matmul moving free dim = one full PSUM bank
TAILC = 256             # tail chunk width (212 real + 44 pad cols)
CS_PAD = 24 * CHUNK + TAILC  # padded classes per core = 12544
ASUB = 3                # psum banks per tile consumed by ScalarE
GROUP = 4               # psum banks (512-col chunks) per full tile
NG_FULL = 6             # full groups of 4 chunks; + 1 tail group of 1 chunk
GCOLS = GROUP * CHUNK   # 2048 logit columns per full group
PB = 128                # partitions
KSUB = D // PB          # 4 contraction subtiles of 128
BBLK = B // PB          # 4 batch blocks
NG = NG_FULL + 1        # total groups per core
N_WARM = 16             # PE warm-up matmuls issued while the first DMAs land

ACT_COLS = ASUB * CHUNK       # 1536 cols on ScalarE
XW = B + GCOLS                # wx0 packed width (x | w0)
WR = GCOLS + TAILC            # wrest packed width (w5 | wtail)

# Schraudolph exp-by-bitcast constants (fp32): exp(x) ~= bitcast_f32(
#   rint(A*x + B0)), mean-centered over the mantissa sawtooth.
SCH_A = float(2.0**23 / math.log(2.0))                   # 12102203.1616
SCH_EG = 1.040684490502804                               # E[(1+f)2^-f]
SCH_B0 = (127.0 - math.log(SCH_EG) / math.log(2.0)) * 2.0**23
SCH_BEFF = SCH_B0 - SCH_A * SHIFT                        # folds the shift

F32 = mybir.dt.float32
I32 = mybir.dt.int32
BF16 = mybir.dt.bfloat16
FP8 = mybir.dt.float8e4
NP_FP8 = mybir.dt.np(FP8)


def build_nc(ncores: int = NCORES):
    """Build the SPMD Bass graph."""
    nc = bacc.Bacc(
        "TRN2",
        target_bir_lowering=False,
        debug=False,
        num_devices=ncores,
    )

    # host-packed operands: per-partition-contiguous SBUF layouts
    wx0_ext = nc.dram_tensor("wx0", [PB, KSUB, XW], FP8, kind="ExternalInput")
    wmid_ext = nc.dram_tensor(
        "wmid", [4 * PB, KSUB, GCOLS], FP8, kind="ExternalInput"
    )
    wrest_ext = nc.dram_tensor("wrest", [PB, KSUB, WR], FP8, kind="ExternalInput")
    pSa_ext = nc.dram_tensor("pSa", [PB, BBLK, NG], F32, kind="ExternalOutput")
    pSv_ext = nc.dram_tensor("pSv", [PB, BBLK, NG], F32, kind="ExternalOutput")

    with tile.TileContext(nc) as tc:
        with (
            tc.tile_pool(name="const", bufs=1) as cpool,
            tc.tile_pool(name="wpool", bufs=4) as wpool,
            tc.tile_pool(name="dpool", bufs=3) as dpool,
            tc.tile_pool(name="ipool", bufs=3) as ipool,
        ):
            # ALL loads on the sync HWDGE ring, in consumption order (one
            # ring sprays all 16 SDMA engines; FIFO order == arrival order;
            # each trigger costs ~650ns of sequencer time).
            # k-halves land in SEPARATE tiles: Tile tracks writes at tile
            # granularity, so a shared tile would make the first matmul wait
            # for BOTH halves (~2.5us later)
            wx0a = cpool.tile([PB, 2, XW], FP8)
            wx0b = cpool.tile([PB, 2, XW], FP8)
            nc.sync.dma_start(out=wx0a, in_=wx0_ext[:, 0:2, :])
            nc.sync.dma_start(out=wx0b, in_=wx0_ext[:, 2:4, :])
            wmids = []
            for g in range(4):
                wt = wpool.tile([PB, KSUB, GCOLS], FP8, name="wt", tag="w")
                nc.sync.dma_start(out=wt, in_=wmid_ext[g * PB : (g + 1) * PB, :, :])
                wmids.append(wt)
            wrest = cpool.tile([PB, KSUB, WR], FP8)
            nc.sync.dma_start(out=wrest, in_=wrest_ext[:])

            # per-(k2) x and group-0 weight APs live in the wx0 halves;
            # groups 1-4 in wpool, 5 + tail in wrest
            wx0h = [wx0a, wx0b]

            def xap(k2, bb):
                return wx0h[k2][:, :, bb * PB : (bb + 1) * PB]

            def wap(g, k2, lo, hi):
                if g == 0:
                    return wx0h[k2][:, :, B + lo : B + hi]
                if g < 5:
                    return wmids[g - 1][:, 2 * k2 : 2 * k2 + 2, lo:hi]
                if g == 5:
                    return wrest[:, 2 * k2 : 2 * k2 + 2, lo:hi]
                return wrest[:, 2 * k2 : 2 * k2 + 2, GCOLS + lo : GCOLS + hi]

            # warm-up operand first so the PE can start immediately
            warm = cpool.tile([PB, 256], BF16)
            nc.vector.memset(warm, 0.0)

            # constants
            negs = cpool.tile([PB, 1], F32)
            nc.vector.memset(negs, -SHIFT)

            # per-row partial sums, one tile per engine (sharing one tile
            # creates a false cross-engine WAW serialization)
            pSa = cpool.tile([PB, BBLK, NG], F32)           # ScalarE accum
            pSv = cpool.tile([PB, BBLK, NG], F32)           # VectorE sums

            # preload the Exp activation table off the critical path
            dumdum = cpool.tile([PB, 1], BF16)
            nc.scalar.activation(
                out=dumdum, in_=negs,
                func=mybir.ActivationFunctionType.Exp, bias=negs, scale=1.0,
            )

            with (
                tc.tile_pool(name="psa", bufs=2, space="PSUM") as pool_a,
                tc.tile_pool(name="psv", bufs=2, space="PSUM") as pool_v,
            ):
                # PE warm-up: dependency-free matmuls so the HAM clock gate
                # is released by the time the first weight tiles arrive.
                ones_bf = nc.const_aps.aps[(BF16, 1.0)]
                warm_ps = pool_a.tile(
                    [PB, ASUB, CHUNK], F32, name="warm_ps", tag="psa",
                )
                for _ in range(N_WARM):
                    nc.tensor.matmul(
                        out=warm_ps[0:1, 0, :256], lhsT=ones_bf, rhs=warm,
                        start=True, stop=True,
                    )

                # full groups in DMA-arrival order; tail tiles slotted where
                # VectorE has slack, one kept last so the ending is short
                tiles = [(g, bb) for g in range(3) for bb in range(BBLK)]
                tiles += [(NG - 1, 0)]
                tiles += [(3, bb) for bb in range(BBLK)]
                tiles += [(NG - 1, 1)]
                tiles += [(4, bb) for bb in range(BBLK)]
                tiles += [(NG - 1, 2)]
                tiles += [(5, bb) for bb in range(BBLK)]
                tiles += [(NG - 1, 3)]
                for g, bb in tiles:
                    if g < NG_FULL:
                        # ScalarE's 3 banks + VectorE's 1 bank, disjoint
                        # PSUM tiles so the consumers never serialize
                        ps_a = pool_a.tile(
                            [PB, ASUB, CHUNK], F32, name="ps_a", tag="psa",
                        )
                        ps_v = pool_v.tile(
                            [PB, CHUNK], F32, name="ps_v", tag="psv",
                        )
                        for k2 in range(KSUB // 2):
                            for sub in range(GROUP):
                                out_ap = (
                                    ps_a[:, sub : sub + 1, :]
                                    if sub < ASUB
                                    else ps_v[:, 0:CHUNK].unsqueeze(1)
                                )
                                nc.tensor.matmul(
                                    out=out_ap,
                                    lhsT=xap(k2, bb),
                                    rhs=wap(
                                        g, k2, sub * CHUNK, (sub + 1) * CHUNK
                                    ),
                                    start=(k2 == 0),
                                    stop=(k2 == KSUB // 2 - 1),
                                    perf_mode=mybir.MatmulPerfMode.DoubleRow,
                                )
                        # ScalarE: exp(l - SHIFT) over the 3-bank tile
                        flat = ps_a.rearrange("p s c -> p (s c)")
                        dump = dpool.tile(
                            [PB, ACT_COLS], BF16, name="dump", tag="dump",
                        )
                        nc.scalar.activation(
                            out=dump,
                            in_=flat,
                            func=mybir.ActivationFunctionType.Exp,
                            bias=negs,
                            scale=1.0,
                            accum_out=pSa[:, bb, g : g + 1],
                        )
                        # VectorE: Schraudolph exp over the 1-bank tile,
                        # summed via the int32->fp32 bitcast
                        idump = ipool.tile(
                            [PB, CHUNK], I32, name="idump", tag="idump",
                        )
                        nc.vector.tensor_scalar(
                            out=idump,
                            in0=ps_v,
                            scalar1=SCH_A,
                            scalar2=SCH_BEFF,
                            op0=mybir.AluOpType.mult,
                            op1=mybir.AluOpType.add,
                        )
                        nc.vector.tensor_reduce(
                            pSv[:, bb, g : g + 1],
                            idump.bitcast(F32),
                            axis=mybir.AxisListType.X,
                            op=mybir.AluOpType.add,
                        )
                    else:
                        # tail tile: one bank; bb 0-2 on VectorE, the final
                        # one (bb=3) on ScalarE, which is idle by then --
                        # its exact-exp pads are corrected separately
                        ps_v = pool_v.tile(
                            [PB, CHUNK], F32, name="ps_vt", tag="psv",
                        )
                        for k2 in range(KSUB // 2):
                            nc.tensor.matmul(
                                out=ps_v[:, 0:TAILC].unsqueeze(1),
                                lhsT=xap(k2, bb),
                                rhs=wap(g, k2, 0, TAILC),
                                start=(k2 == 0),
                                stop=(k2 == KSUB // 2 - 1),
                                perf_mode=mybir.MatmulPerfMode.DoubleRow,
                            )
                        if bb < 3:
                            idump = ipool.tile(
                                [PB, TAILC], I32, name="idumpt", tag="idump",
                                padded_shape=[PB, CHUNK],
                            )
                            nc.vector.tensor_scalar(
                                out=idump,
                                in0=ps_v[:, 0:TAILC],
                                scalar1=SCH_A,
                                scalar2=SCH_BEFF,
                                op0=mybir.AluOpType.mult,
                                op1=mybir.AluOpType.add,
                            )
                            nc.vector.tensor_reduce(
                                pSv[:, bb, NG_FULL : NG_FULL + 1],
                                idump.bitcast(F32),
                                axis=mybir.AxisListType.X,
                                op=mybir.AluOpType.add,
                            )
                        else:
                            dump = dpool.tile(
                                [PB, TAILC], BF16, name="dumpt", tag="dump",
                                padded_shape=[PB, ACT_COLS],
                            )
                            nc.scalar.activation(
                                out=dump,
                                in_=ps_v[:, 0:TAILC],
                                func=mybir.ActivationFunctionType.Exp,
                                bias=negs,
                                scale=1.0,
                                accum_out=pSa[:, bb, NG_FULL : NG_FULL + 1],
                            )

            # raw per-engine partials out; host does the final sums (the
            # device reduce chain would sit on the critical path)
            nc.sync.dma_start(out=pSv_ext[:], in_=pSv)
            nc.sync.dma_start(out=pSa_ext[:], in_=pSa)

    nc.finalize()
    return nc


def prepare_inputs(x, weight, label, ncores: int = NCORES):
    """Host-side prep: normalize, G-scale, cast fp8, pack to SBUF layouts.

    Returns (in_maps, lc2) where lc2[p, j] = SHIFT - S*cos(x_b, w_label_b)
    for b = j*128 + p."""
    x = np.asarray(x, dtype=np.float32)
    weight = np.asarray(weight, dtype=np.float32)
    label = np.asarray(label).astype(np.int64)

    xn = x / np.maximum(
        np.sqrt(np.einsum("bd,bd->b", x, x, dtype=np.float64))[:, None], EPS
    ).astype(np.float32)
    wnorm = np.sqrt(np.einsum("cd,cd->c", weight, weight, dtype=np.float64))
    wn = weight / np.maximum(wnorm[:, None], EPS).astype(np.float32)

    # label cosine computed on host in f64 (exact vs fp32 reference)
    wl = wn[label]  # [B, D]
    label_cos = np.einsum("bd,bd->b", xn.astype(np.float64), wl.astype(np.float64))
    lc2 = (SHIFT - S * label_cos).astype(np.float64)  # [B]
    lc2_pj = np.ascontiguousarray(lc2.reshape(BBLK, PB).T)  # [128, BBLK]

    x8 = (G * xn).astype(NP_FP8)          # [B, D]
    w8 = (G * wn).astype(NP_FP8)          # [C, D]
    # xnt[p, ks, b] = x8[b, ks*128 + p]
    xp = x8.reshape(B, KSUB, PB).transpose(2, 1, 0)  # [128, 4, 512]

    in_maps = []
    for i in range(ncores):
        wp = np.zeros((CS_PAD, D), dtype=NP_FP8)
        wp[:CS] = w8[i * CS : (i + 1) * CS]
        # group g block: [p, ks, col] = wp[g*2048 + col, ks*128 + p]
        main = (
            wp[: NG_FULL * GCOLS]
            .reshape(NG_FULL, GCOLS, KSUB, PB)
            .transpose(0, 3, 2, 1)  # [6, 128, 4, 2048]
        )
        tail = wp[NG_FULL * GCOLS :].reshape(TAILC, KSUB, PB).transpose(2, 1, 0)
        wx0 = np.concatenate([xp, main[0]], axis=2)          # [128, 4, 2560]
        wmid = main[1:5].reshape(4 * PB, KSUB, GCOLS)
        wrest = np.concatenate([main[5], tail], axis=2)      # [128, 4, 2304]
        in_maps.append(
            {
                "wx0": np.ascontiguousarray(wx0),
                "wmid": np.ascontiguousarray(wmid),
                "wrest": np.ascontiguousarray(wrest),
            }
        )
    return in_maps, lc2_pj


_NC_CACHE = {}


def _get_nc():
    if "nc" not in _NC_CACHE:
        _NC_CACHE["nc"] = build_nc()
    return _NC_CACHE["nc"]


def _install_ntff_hook():
    """The agent image's antenv lacks axon_hooks; shim it so trace=True can
    capture NTFF profiles via the ctypes hook in trn_agent_boot."""
    import sys
    import types

    try:
        from antenv.axon_hooks import get_axon_ntff_profile_hook  # noqa: F401
        return
    except ImportError:
        pass
    mod = types.ModuleType("antenv.axon_hooks")
    _state = {"hook": None}
    mod.set_axon_ntff_profile_hook = lambda h: _state.__setitem__("hook", h)
    mod.get_axon_ntff_profile_hook = lambda: _state["hook"]
    sys.modules["antenv.axon_hooks"] = mod
    import antenv

    antenv.axon_hooks = mod
    from trn_agent_boot.trn_boot import _ntff_profile_via_ctypes

    mod.set_axon_ntff_profile_hook(
        _ntff_profile_via_ctypes("/opt/axon/libaxon_pjrt.so")
    )
    # keep trace artifacts local (no external upload from this sandbox)
    import concourse.bass_utils as bu

    bu.upload_artifacts = lambda tmpdir: tmpdir


def finish_loss(results, lc2_pj):
    """Host epilogue: sum the 8 per-core partials, remove the exact
    zero-pad contribution, log, add label term, mean."""
    Z = np.zeros((PB, BBLK), dtype=np.float64)
    for r in results:
        a = r["pSa"].astype(np.float64)  # [128, 4, 7]
        v = r["pSv"].astype(np.float64)
        Z += a[:, :, :NG_FULL].sum(axis=2) + v[:, :, :NG_FULL].sum(axis=2)
        Z[:, 0:3] += v[:, 0:3, NG_FULL]   # tail tiles bb 0-2 (VectorE)
        Z[:, 3] += a[:, 3, NG_FULL]       # tail tile bb 3 (ScalarE)
    # pads: tail-tile cols 212..255 are zero logits. bb 0-2 rode the
    # VectorE Schraudolph path (bit-deterministic image of
    # rint(fp32(SCH_BEFF))); bb 3 rode ScalarE (exact exp(-SHIFT)).
    n_pad = CS_PAD - CS                      # 44
    pad_img = np.int32(np.rint(np.float32(0.0) * np.float32(SCH_A)
                               + np.float32(SCH_BEFF)))
    pad_val = float(np.frombuffer(pad_img.tobytes(), dtype=np.float32)[0])
    Z[:, 0:3] -= NCORES * n_pad * pad_val
    Z[:, 3] -= NCORES * n_pad * math.exp(-SHIFT)
    loss = float((np.log(Z) + lc2_pj).mean())
    return np.float32(loss)


def run(x, weight, label, trace=False):
    """Returns (loss_scalar, BassKernelResults)."""
    if trace:
        _install_ntff_hook()
    nc = _get_nc()
    in_maps, lc2_pj = prepare_inputs(x, weight, label)
    res = run_bass_kernel_spmd(
        nc, in_maps, core_ids=list(range(NCORES)), trace=trace
    )
    loss = finish_loss(res.results, lc2_pj)
    return loss, res


def kernel(x, weight, label, batch=None, **_ignored):
    loss, _ = run(x, weight, label, trace=False)
    return np.asarray(loss, dtype=np.float32)


# revision 8
# speedup vs baseline: 1.1881x; 1.1881x over previous
"""ArcFace loss kernel for 8 TRN2 NeuronCores (vocab/tensor-parallel).

reference:
    xn = normalize(x)               # [B, D]
    wn = normalize(weight)          # [C, D]
    logits = 64 * xn @ wn.T         # [B, C]
    loss = mean(CE(logits, label))

Strategy: shard classes C=100000 over 8 cores (12500 each, zero-padded to
12544 = 24*512 + 256). Host prepares normalized fp8(e4m3) operands scaled
by G=8 (so device cosines are 64*cos), pre-packed in the exact SBUF tile
layout so every weight-group DMA is 8KB-contiguous per partition. Each core
computes its logit shard with TensorE fp8 DoubleRow matmuls (K=256/op) into
fp32 PSUM.

The sum-of-exp over each PSUM tile is split across two engines so the PE
stream (8 x 216ns matmuls = ~1.73us/tile) is the only bottleneck:
  - ScalarE: one fused exp(l - SHIFT) activation over a dedicated 3-bank
    PSUM tile (1536 cols) with row-accumulate (~1.54us + 182ns drain).
  - VectorE: Schraudolph exp on a dedicated 1-bank PSUM tile (512 cols):
    i32 = rint(A*l + Beff) via one tensor_scalar (fp32 mul-add, HW
    verified round-to-nearest), whose int32 bits ARE the fp32 exp image;
    a tensor_reduce over the bitcast sums it (~1.4us total). The
    mean-centering constant absorbs the (1+f)2^-f mantissa sawtooth
    (E[g]=1.040684); residual loss error ~1e-4 relative.
  The two consumers read DISJOINT PSUM tiles (separate pools): Tile's
  access tracker serializes cross-engine readers of one tile, which
  otherwise stalls the PE every other tile and trips HAM re-throttling.

DMA: every HWDGE trigger costs ~650ns of sequencer time, serially, so
transfers are packed to minimize trigger count ahead of the critical
path: x and w0 ride one dram tensor (wx0, k-halved so the first matmuls
start ~1.2us earlier), w5 and the tail share another. All on the sync
ring in exact consumption order; one ring already sprays all 16 SDMA
engines at ~420GB/s aggregate.

Tail tiles (256 cols) go entirely to VectorE (ScalarE runs at zero slack);
the 44 zero-pad columns contribute a bit-deterministic Schraudolph image
of exp(-SHIFT) that the host subtracts exactly. Each core returns per-row
partial sums [128, 4]; the host gathers the 8 cores and finishes
loss = mean(log Z + SHIFT - 64*cos_label) with host-exact label cosines.
"""

import math
import numpy as np

import concourse.mybir as mybir
import concourse.tile as tile
from concourse import bacc
from concourse.bass_utils import run_bass_kernel_spmd

# Problem constants (hardcoded per harness contract).
B = 512
D = 512
C = 100000
S = 64.0
SHIFT = 20.0  # logsumexp shift; keeps Z ~1e-2
EPS = 1e-12
G = 8.0      # fp8 pre-scale on both operands: device cos' = G^2 * cos
NCORES = 8
CS = C // NCORES        # true classes per core = 12500
CHUNK = 512             # matmul moving free dim = one full PSUM bank
TAILC = 256             # tail chunk width (212 real + 44 pad cols)
CS_PAD = 24 * CHUNK + TAILC  # padded classes per core = 12544
ASUB = 3                # psum banks per tile consumed by ScalarE
GROUP = 4               # psum banks (512-col chunks) per full tile
NG_FULL = 6             # full groups of 4 chunks; + 1 tail group of 1 chunk
GCOLS = GROUP * CHUNK   # 2048 logit columns per full group
PB = 128                # partitions
KSUB = D // PB          # 4 contraction subtiles of 128
BBLK = B // PB          # 4 batch blocks
NG = NG_FULL + 1        # total groups per core
N_WARM = 14             # PE warm-up matmuls issued while the first DMAs land

ACT_COLS = ASUB * CHUNK       # 1536 cols on ScalarE
XW = B + GCOLS                # wx0 packed width (x | w0)
WR = GCOLS + TAILC            # wrest packed width (w5 | wtail)

# Schraudolph exp-by-bitcast constants (fp32): exp(x) ~= bitcast_f32(
#   rint(A*x + B0)), mean-centered over the mantissa sawtooth.
SCH_A = float(2.0**23 / math.log(2.0))                   # 12102203.1616
SCH_EG = 1.040684490502804                               # E[(1+f)2^-f]
SCH_B0 = (127.0 - math.log(SCH_EG) / math.log(2.0)) * 2.0**23
SCH_BEFF = SCH_B0 - SCH_A * SHIFT                        # folds the shift

F32 = mybir.dt.float32
I32 = mybir.dt.int32
BF16 = mybir.dt.bfloat16
FP8 = mybir.dt.float8e4
NP_FP8 = mybir.dt.np(FP8)


def build_nc(ncores: int = NCORES):
    """Build the SPMD Bass graph."""
    nc = bacc.Bacc(
        "TRN2",
        target_bir_lowering=False,
        debug=False,
        num_devices=ncores,
    )

    # host-packed operands: per-partition-contiguous SBUF layouts
    wx0_ext = nc.dram_tensor("wx0", [PB, KSUB, XW], FP8, kind="ExternalInput")
    wmid_ext = nc.dram_tensor(
        "wmid", [4 * PB, KSUB, GCOLS], FP8, kind="ExternalInput"
    )
    wrest_ext = nc.dram_tensor("wrest", [PB, KSUB, WR], FP8, kind="ExternalInput")
    pSa_ext = nc.dram_tensor("pSa", [PB, BBLK, NG], F32, kind="ExternalOutput")
    pSv_ext = nc.dram_tensor("pSv", [PB, BBLK, NG], F32, kind="ExternalOutput")

    with tile.TileContext(nc) as tc:
        with (
            tc.tile_pool(name="const", bufs=1) as cpool,
            tc.tile_pool(name="wpool", bufs=4) as wpool,
            tc.tile_pool(name="dpool", bufs=3) as dpool,
            tc.tile_pool(name="ipool", bufs=3) as ipool,
        ):
            # ALL loads on the sync HWDGE ring, in consumption order (one
            # ring sprays all 16 SDMA engines; FIFO order == arrival order;
            # each trigger costs ~650ns of sequencer time).
            # k-halves land in SEPARATE tiles: Tile tracks writes at tile
            # granularity, so a shared tile would make the first matmul wait
            # for BOTH halves (~2.5us later)
            wx0a = cpool.tile([PB, 2, XW], FP8)
            wx0b = cpool.tile([PB, 2, XW], FP8)
            nc.sync.dma_start(out=wx0a, in_=wx0_ext[:, 0:2, :])
            nc.sync.dma_start(out=wx0b, in_=wx0_ext[:, 2:4, :])
            wmids = []
            for g in range(4):
                wt = wpool.tile([PB, KSUB, GCOLS], FP8, name="wt", tag="w")
                nc.sync.dma_start(out=wt, in_=wmid_ext[g * PB : (g + 1) * PB, :, :])
                wmids.append(wt)
            wrest = cpool.tile([PB, KSUB, WR], FP8)
            nc.sync.dma_start(out=wrest, in_=wrest_ext[:])

            # per-(k2) x and group-0 weight APs live in the wx0 halves;
            # groups 1-4 in wpool, 5 + tail in wrest
            wx0h = [wx0a, wx0b]

            def xap(k2, bb):
                return wx0h[k2][:, :, bb * PB : (bb + 1) * PB]

            def wap(g, k2, lo, hi):
                if g == 0:
                    return wx0h[k2][:, :, B + lo : B + hi]
                if g < 5:
                    return wmids[g - 1][:, 2 * k2 : 2 * k2 + 2, lo:hi]
                if g == 5:
                    return wrest[:, 2 * k2 : 2 * k2 + 2, lo:hi]
                return wrest[:, 2 * k2 : 2 * k2 + 2, GCOLS + lo : GCOLS + hi]

            # warm-up operand first so the PE can start immediately
            warm = cpool.tile([PB, 256], BF16)
            nc.vector.memset(warm, 0.0)

            # constants
            negs = cpool.tile([PB, 1], F32)
            nc.vector.memset(negs, -SHIFT)

            # per-row partial sums, one tile per engine (sharing one tile
            # creates a false cross-engine WAW serialization)
            pSa = cpool.tile([PB, BBLK, NG], F32)           # ScalarE accum
            pSv = cpool.tile([PB, BBLK, NG], F32)           # VectorE sums

            # preload the Exp activation table off the critical path
            dumdum = cpool.tile([PB, 1], BF16)
            nc.scalar.activation(
                out=dumdum, in_=negs,
                func=mybir.ActivationFunctionType.Exp, bias=negs, scale=1.0,
            )

            with (
                tc.tile_pool(name="psa", bufs=2, space="PSUM") as pool_a,
                tc.tile_pool(name="psv", bufs=2, space="PSUM") as pool_v,
            ):
                # PE warm-up: dependency-free matmuls so the HAM clock gate
                # is released by the time the first weight tiles arrive.
                ones_bf = nc.const_aps.aps[(BF16, 1.0)]
                warm_ps = pool_a.tile(
                    [PB, ASUB, CHUNK], F32, name="warm_ps", tag="psa",
                )
                for _ in range(N_WARM):
                    nc.tensor.matmul(
                        out=warm_ps[0:1, 0, :256], lhsT=ones_bf, rhs=warm,
                        start=True, stop=True,
                    )

                # full groups in DMA-arrival order; tail tiles slotted where
                # VectorE has slack, one kept last so the ending is short
                tiles = [(g, bb) for g in range(3) for bb in range(BBLK)]
                tiles += [(NG - 1, 0)]
                tiles += [(3, bb) for bb in range(BBLK)]
                tiles += [(NG - 1, 1)]
                tiles += [(4, bb) for bb in range(BBLK)]
                tiles += [(NG - 1, 2)]
                tiles += [(5, bb) for bb in range(BBLK)]
                tiles += [(NG - 1, 3)]
                for g, bb in tiles:
                    if g < NG_FULL:
                        # ScalarE's 3 banks + VectorE's 1 bank, disjoint
                        # PSUM tiles so the consumers never serialize
                        ps_a = pool_a.tile(
                            [PB, ASUB, CHUNK], F32, name="ps_a", tag="psa",
                        )
                        ps_v = pool_v.tile(
                            [PB, CHUNK], F32, name="ps_v", tag="psv",
                        )
                        for k2 in range(KSUB // 2):
                            for sub in range(GROUP):
                                out_ap = (
                                    ps_a[:, sub : sub + 1, :]
                                    if sub < ASUB
                                    else ps_v[:, 0:CHUNK].unsqueeze(1)
                                )
                                nc.tensor.matmul(
                                    out=out_ap,
                                    lhsT=xap(k2, bb),
                                    rhs=wap(
                                        g, k2, sub * CHUNK, (sub + 1) * CHUNK
                                    ),
                                    start=(k2 == 0),
                                    stop=(k2 == KSUB // 2 - 1),
                                    perf_mode=mybir.MatmulPerfMode.DoubleRow,
                                )
                        # ScalarE: exp(l - SHIFT) over the 3-bank tile
                        flat = ps_a.rearrange("p s c -> p (s c)")
                        dump = dpool.tile(
                            [PB, ACT_COLS], BF16, name="dump", tag="dump",
                        )
                        nc.scalar.activation(
                            out=dump,
                            in_=flat,
                            func=mybir.ActivationFunctionType.Exp,
                            bias=negs,
                            scale=1.0,
                            accum_out=pSa[:, bb, g : g + 1],
                        )
                        # VectorE: Schraudolph exp over the 1-bank tile,
                        # summed via the int32->fp32 bitcast
                        idump = ipool.tile(
                            [PB, CHUNK], I32, name="idump", tag="idump",
                        )
                        nc.vector.tensor_scalar(
                            out=idump,
                            in0=ps_v,
                            scalar1=SCH_A,
                            scalar2=SCH_BEFF,
                            op0=mybir.AluOpType.mult,
                            op1=mybir.AluOpType.add,
                        )
                        nc.vector.tensor_reduce(
                            pSv[:, bb, g : g + 1],
                            idump.bitcast(F32),
                            axis=mybir.AxisListType.X,
                            op=mybir.AluOpType.add,
                        )
                    else:
                        # tail tile: one bank, entirely VectorE (idle at the
                        # end while ScalarE drains its last full-tile EXP)
                        ps_v = pool_v.tile(
                            [PB, CHUNK], F32, name="ps_vt", tag="psv",
                        )
                        for k2 in range(KSUB // 2):
                            nc.tensor.matmul(
                                out=ps_v[:, 0:TAILC].unsqueeze(1),
                                lhsT=xap(k2, bb),
                                rhs=wap(g, k2, 0, TAILC),
                                start=(k2 == 0),
                                stop=(k2 == KSUB // 2 - 1),
                                perf_mode=mybir.MatmulPerfMode.DoubleRow,
                            )
                        idump = ipool.tile(
                            [PB, TAILC], I32, name="idumpt", tag="idump",
                            padded_shape=[PB, CHUNK],
                        )
                        nc.vector.tensor_scalar(
                            out=idump,
                            in0=ps_v[:, 0:TAILC],
                            scalar1=SCH_A,
                            scalar2=SCH_BEFF,
                            op0=mybir.AluOpType.mult,
                            op1=mybir.AluOpType.add,
                        )
                        nc.vector.tensor_reduce(
                            pSv[:, bb, NG_FULL : NG_FULL + 1],
                            idump.bitcast(F32),
                            axis=mybir.AxisListType.X,
                            op=mybir.AluOpType.add,
                        )

            # raw per-engine partials out; host does the final sums (the
            # device reduce chain would sit on the critical path)
            nc.sync.dma_start(out=pSv_ext[:], in_=pSv)
            nc.sync.dma_start(out=pSa_ext[:], in_=pSa)

    nc.finalize()
    return nc


def prepare_inputs(x, weight, label, ncores: int = NCORES):
    """Host-side prep: normalize, G-scale, cast fp8, pack to SBUF layouts.

    Returns (in_maps, lc2) where lc2[p, j] = SHIFT - S*cos(x_b, w_label_b)
    for b = j*128 + p."""
    x = np.asarray(x, dtype=np.float32)
    weight = np.asarray(weight, dtype=np.float32)
    label = np.asarray(label).astype(np.int64)

    xn = x / np.maximum(
        np.sqrt(np.einsum("bd,bd->b", x, x, dtype=np.float64))[:, None], EPS
    ).astype(np.float32)
    wnorm = np.sqrt(np.einsum("cd,cd->c", weight, weight, dtype=np.float64))
    wn = weight / np.maximum(wnorm[:, None], EPS).astype(np.float32)

    # label cosine computed on host in f64 (exact vs fp32 reference)
    wl = wn[label]  # [B, D]
    label_cos = np.einsum("bd,bd->b", xn.astype(np.float64), wl.astype(np.float64))
    lc2 = (SHIFT - S * label_cos).astype(np.float64)  # [B]
    lc2_pj = np.ascontiguousarray(lc2.reshape(BBLK, PB).T)  # [128, BBLK]

    x8 = (G * xn).astype(NP_FP8)          # [B, D]
    w8 = (G * wn).astype(NP_FP8)          # [C, D]
    # xnt[p, ks, b] = x8[b, ks*128 + p]
    xp = x8.reshape(B, KSUB, PB).transpose(2, 1, 0)  # [128, 4, 512]

    in_maps = []
    for i in range(ncores):
        wp = np.zeros((CS_PAD, D), dtype=NP_FP8)
        wp[:CS] = w8[i * CS : (i + 1) * CS]
        # group g block: [p, ks, col] = wp[g*2048 + col, ks*128 + p]
        main = (
            wp[: NG_FULL * GCOLS]
            .reshape(NG_FULL, GCOLS, KSUB, PB)
            .transpose(0, 3, 2, 1)  # [6, 128, 4, 2048]
        )
        tail = wp[NG_FULL * GCOLS :].reshape(TAILC, KSUB, PB).transpose(2, 1, 0)
        wx0 = np.concatenate([xp, main[0]], axis=2)          # [128, 4, 2560]
        wmid = main[1:5].reshape(4 * PB, KSUB, GCOLS)
        wrest = np.concatenate([main[5], tail], axis=2)      # [128, 4, 2304]
        in_maps.append(
            {
                "wx0": np.ascontiguousarray(wx0),
                "wmid": np.ascontiguousarray(wmid),
                "wrest": np.ascontiguousarray(wrest),
            }
        )
    return in_maps, lc2_pj


_NC_CACHE = {}


def _get_nc():
    if "nc" not in _NC_CACHE:
        _NC_CACHE["nc"] = build_nc()
    return _NC_CACHE["nc"]


def _install_ntff_hook():
    """The agent image's antenv lacks axon_hooks; shim it so trace=True can
    capture NTFF profiles via the ctypes hook in trn_agent_boot."""
    import sys
    import types

    try:
        from antenv.axon_hooks import get_axon_ntff_profile_hook  # noqa: F401
        return
    except ImportError:
        pass
    mod = types.ModuleType("antenv.axon_hooks")
    _state = {"hook": None}
    mod.set_axon_ntff_profile_hook = lambda h: _state.__setitem__("hook", h)
    mod.get_axon_ntff_profile_hook = lambda: _state["hook"]
    sys.modules["antenv.axon_hooks"] = mod
    import antenv

    antenv.axon_hooks = mod
    from trn_agent_boot.trn_boot import _ntff_profile_via_ctypes

    mod.set_axon_ntff_profile_hook(
        _ntff_profile_via_ctypes("/opt/axon/libaxon_pjrt.so")
    )
    # keep trace artifacts local (no external upload from this sandbox)
    import concourse.bass_utils as bu

    bu.upload_artifacts = lambda tmpdir: tmpdir


def finish_loss(results, lc2_pj):
    """Host epilogue: sum the 8 per-core partials, remove the exact
    zero-pad contribution, log, add label term, mean."""
    Z = np.zeros((PB, BBLK), dtype=np.float64)
    for r in results:
        a = r["pSa"].astype(np.float64)  # [128, 4, 7]
        v = r["pSv"].astype(np.float64)
        Z += a[:, :, :NG_FULL].sum(axis=2) + v.sum(axis=2)
    # pads: tail-tile cols 212..255 are zero logits on the VectorE
    # Schraudolph path -> each contributes the bit-deterministic image of
    # rint(fp32(SCH_BEFF)) reinterpreted as fp32
    n_pad = CS_PAD - CS                      # 44
    pad_img = np.int32(np.rint(np.float32(0.0) * np.float32(SCH_A)
                               + np.float32(SCH_BEFF)))
    pad_val = float(np.frombuffer(pad_img.tobytes(), dtype=np.float32)[0])
    Z -= NCORES * n_pad * pad_val
    loss = float((np.log(Z) + lc2_pj).mean())
    return np.float32(loss)


def run(x, weight, label, trace=False):
    """Returns (loss_scalar, BassKernelResults)."""
    if trace:
        _install_ntff_hook()
    nc = _get_nc()
    in_maps, lc2_pj = prepare_inputs(x, weight, label)
    res = run_bass_kernel_spmd(
        nc, in_maps, core_ids=list(range(NCORES)), trace=trace
    )
    loss = finish_loss(res.results, lc2_pj)
    return loss, res


def kernel(x, weight, label, batch=None, **_ignored):
    loss, _ = run(x, weight, label, trace=False)
    return np.asarray(loss, dtype=np.float32)


# revision 9
# speedup vs baseline: 1.1940x; 1.0050x over previous
"""ArcFace loss kernel for 8 TRN2 NeuronCores (vocab/tensor-parallel).

reference:
    xn = normalize(x)               # [B, D]
    wn = normalize(weight)          # [C, D]
    logits = 64 * xn @ wn.T         # [B, C]
    loss = mean(CE(logits, label))

Strategy: shard classes C=100000 over 8 cores (12500 each, zero-padded to
12544 = 24*512 + 256). Host prepares normalized fp8(e4m3) operands scaled
by G=8 (so device cosines are 64*cos), pre-packed in the exact SBUF tile
layout so every weight-group DMA is 8KB-contiguous per partition. Each core
computes its logit shard with TensorE fp8 DoubleRow matmuls (K=256/op) into
fp32 PSUM.

The sum-of-exp over each PSUM tile is split across two engines so the PE
stream (8 x 216ns matmuls = ~1.73us/tile) is the only bottleneck:
  - ScalarE: one fused exp(l - SHIFT) activation over a dedicated 3-bank
    PSUM tile (1536 cols) with row-accumulate (~1.54us + 182ns drain).
  - VectorE: Schraudolph exp on a dedicated 1-bank PSUM tile (512 cols):
    i32 = rint(A*l + Beff) via one tensor_scalar (fp32 mul-add, HW
    verified round-to-nearest), whose int32 bits ARE the fp32 exp image;
    a tensor_reduce over the bitcast sums it (~1.4us total). The
    mean-centering constant absorbs the (1+f)2^-f mantissa sawtooth
    (E[g]=1.040684); residual loss error ~1e-4 relative.
  The two consumers read DISJOINT PSUM tiles (separate pools): Tile's
  access tracker serializes cross-engine readers of one tile, which
  otherwise stalls the PE every other tile and trips HAM re-throttling.

DMA: every HWDGE trigger costs ~650ns of sequencer time, serially, so
transfers are packed to minimize trigger count ahead of the critical
path: x and w0 ride one dram tensor (wx0, k-halved so the first matmuls
start ~1.2us earlier), w5 and the tail share another. All on the sync
ring in exact consumption order; one ring already sprays all 16 SDMA
engines at ~420GB/s aggregate.

Tail tiles (256 cols) go entirely to VectorE (ScalarE runs at zero slack);
the 44 zero-pad columns contribute a bit-deterministic Schraudolph image
of exp(-SHIFT) that the host subtracts exactly. Each core returns per-row
partial sums [128, 4]; the host gathers the 8 cores and finishes
loss = mean(log Z + SHIFT - 64*cos_label) with host-exact label cosines.
"""

import math
import numpy as np

import concourse.mybir as mybir
import concourse.tile as tile
from concourse import bacc
from concourse.bass_utils import run_bass_kernel_spmd

# Problem constants (hardcoded per harness contract).
B = 512
D = 512
C = 100000
S = 64.0
SHIFT = 20.0  # logsumexp shift; keeps Z ~1e-2
EPS = 1e-12
G = 8.0      # fp8 pre-scale on both operands: device cos' = G^2 * cos
NCORES = 8
CS = C // NCORES        # true classes per core = 12500
CHUNK = 512             # matmul moving free dim = one full PSUM bank
TAILC = 256             # tail chunk width (212 real + 44 pad cols)
CS_PAD = 24 * CHUNK + TAILC  # padded classes per core = 12544
ASUB = 3                # psum banks per tile consumed by ScalarE
GROUP = 4               # psum banks (512-col chunks) per full tile
NG_FULL = 6             # full groups of 4 chunks; + 1 tail group of 1 chunk
GCOLS = GROUP * CHUNK   # 2048 logit columns per full group
PB = 128                # partitions
KSUB = D // PB          # 4 contraction subtiles of 128
BBLK = B // PB          # 4 batch blocks
NG = NG_FULL + 1        # total groups per core
N_WARM = 14             # PE warm-up matmuls issued while the first DMAs land

ACT_COLS = ASUB * CHUNK       # 1536 cols on ScalarE
XW = B + GCOLS                # wx0 packed width (x | w0)
WR = GCOLS + TAILC            # wrest packed width (w5 | wtail)

# Schraudolph exp-by-bitcast constants (fp32): exp(x) ~= bitcast_f32(
#   rint(A*x + B0)), mean-centered over the mantissa sawtooth.
SCH_A = float(2.0**23 / math.log(2.0))                   # 12102203.1616
SCH_EG = 1.040684490502804                               # E[(1+f)2^-f]
SCH_B0 = (127.0 - math.log(SCH_EG) / math.log(2.0)) * 2.0**23
SCH_BEFF = SCH_B0 - SCH_A * SHIFT                        # folds the shift

F32 = mybir.dt.float32
I32 = mybir.dt.int32
BF16 = mybir.dt.bfloat16
FP8 = mybir.dt.float8e4
NP_FP8 = mybir.dt.np(FP8)


def build_nc(ncores: int = NCORES):
    """Build the SPMD Bass graph."""
    nc = bacc.Bacc(
        "TRN2",
        target_bir_lowering=False,
        debug=False,
        num_devices=ncores,
    )

    # host-packed operands: per-partition-contiguous SBUF layouts
    wx0_ext = nc.dram_tensor("wx0", [PB, KSUB, XW], FP8, kind="ExternalInput")
    wmid_ext = nc.dram_tensor(
        "wmid", [4 * PB, KSUB, GCOLS], FP8, kind="ExternalInput"
    )
    wrest_ext = nc.dram_tensor("wrest", [PB, KSUB, WR], FP8, kind="ExternalInput")
    pSa_ext = nc.dram_tensor("pSa", [PB, BBLK, NG], F32, kind="ExternalOutput")
    pSv_ext = nc.dram_tensor("pSv", [PB, BBLK, NG], F32, kind="ExternalOutput")

    with tile.TileContext(nc) as tc:
        with (
            tc.tile_pool(name="const", bufs=1) as cpool,
            tc.tile_pool(name="wpool", bufs=4) as wpool,
            tc.tile_pool(name="dpool", bufs=3) as dpool,
            tc.tile_pool(name="ipool", bufs=3) as ipool,
        ):
            # ALL loads on the sync HWDGE ring, in consumption order (one
            # ring sprays all 16 SDMA engines; FIFO order == arrival order;
            # each trigger costs ~650ns of sequencer time).
            # k-halves land in SEPARATE tiles: Tile tracks writes at tile
            # granularity, so a shared tile would make the first matmul wait
            # for BOTH halves (~2.5us later)
            wx0a = cpool.tile([PB, 2, XW], FP8)
            wx0b = cpool.tile([PB, 2, XW], FP8)
            nc.sync.dma_start(out=wx0a, in_=wx0_ext[:, 0:2, :])
            nc.sync.dma_start(out=wx0b, in_=wx0_ext[:, 2:4, :])
            wmids = []
            for g in range(4):
                wt = wpool.tile([PB, KSUB, GCOLS], FP8, name="wt", tag="w")
                nc.sync.dma_start(out=wt, in_=wmid_ext[g * PB : (g + 1) * PB, :, :])
                wmids.append(wt)
            wrest = cpool.tile([PB, KSUB, WR], FP8)
            nc.sync.dma_start(out=wrest, in_=wrest_ext[:])

            # per-(k2) x and group-0 weight APs live in the wx0 halves;
            # groups 1-4 in wpool, 5 + tail in wrest
            wx0h = [wx0a, wx0b]

            def xap(k2, bb):
                return wx0h[k2][:, :, bb * PB : (bb + 1) * PB]

            def wap(g, k2, lo, hi):
                if g == 0:
                    return wx0h[k2][:, :, B + lo : B + hi]
                if g < 5:
                    return wmids[g - 1][:, 2 * k2 : 2 * k2 + 2, lo:hi]
                if g == 5:
                    return wrest[:, 2 * k2 : 2 * k2 + 2, lo:hi]
                return wrest[:, 2 * k2 : 2 * k2 + 2, GCOLS + lo : GCOLS + hi]

            # warm-up operand first so the PE can start immediately
            warm = cpool.tile([PB, 256], BF16)
            nc.vector.memset(warm, 0.0)

            # constants
            negs = cpool.tile([PB, 1], F32)
            nc.vector.memset(negs, -SHIFT)

            # per-row partial sums, one tile per engine (sharing one tile
            # creates a false cross-engine WAW serialization)
            pSa = cpool.tile([PB, BBLK, NG], F32)           # ScalarE accum
            pSv = cpool.tile([PB, BBLK, NG], F32)           # VectorE sums

            # preload the Exp activation table off the critical path
            dumdum = cpool.tile([PB, 1], BF16)
            nc.scalar.activation(
                out=dumdum, in_=negs,
                func=mybir.ActivationFunctionType.Exp, bias=negs, scale=1.0,
            )

            with (
                tc.tile_pool(name="psa", bufs=2, space="PSUM") as pool_a,
                tc.tile_pool(name="psv", bufs=2, space="PSUM") as pool_v,
            ):
                # PE warm-up: dependency-free matmuls so the HAM clock gate
                # is released by the time the first weight tiles arrive.
                ones_bf = nc.const_aps.aps[(BF16, 1.0)]
                warm_ps = pool_a.tile(
                    [PB, ASUB, CHUNK], F32, name="warm_ps", tag="psa",
                )
                for _ in range(N_WARM):
                    nc.tensor.matmul(
                        out=warm_ps[0:1, 0, :256], lhsT=ones_bf, rhs=warm,
                        start=True, stop=True,
                    )

                # full groups in DMA-arrival order; tail tiles slotted where
                # VectorE has slack, one kept last so the ending is short
                tiles = [(g, bb) for g in range(2) for bb in range(BBLK)]
                tiles += [(2, 0), (NG - 1, 0), (2, 1), (2, 2), (2, 3)]
                tiles += [(3, 0), (NG - 1, 1), (3, 1), (3, 2), (3, 3)]
                tiles += [(4, 0), (NG - 1, 2), (4, 1), (4, 2), (4, 3)]
                tiles += [(5, bb) for bb in range(BBLK)]
                tiles += [(NG - 1, 3)]
                for g, bb in tiles:
                    if g < NG_FULL:
                        # ScalarE's 3 banks + VectorE's 1 bank, disjoint
                        # PSUM tiles so the consumers never serialize
                        ps_a = pool_a.tile(
                            [PB, ASUB, CHUNK], F32, name="ps_a", tag="psa",
                        )
                        ps_v = pool_v.tile(
                            [PB, CHUNK], F32, name="ps_v", tag="psv",
                        )
                        for k2 in range(KSUB // 2):
                            for sub in range(GROUP):
                                out_ap = (
                                    ps_a[:, sub : sub + 1, :]
                                    if sub < ASUB
                                    else ps_v[:, 0:CHUNK].unsqueeze(1)
                                )
                                nc.tensor.matmul(
                                    out=out_ap,
                                    lhsT=xap(k2, bb),
                                    rhs=wap(
                                        g, k2, sub * CHUNK, (sub + 1) * CHUNK
                                    ),
                                    start=(k2 == 0),
                                    stop=(k2 == KSUB // 2 - 1),
                                    perf_mode=mybir.MatmulPerfMode.DoubleRow,
                                )
                        # ScalarE: exp(l - SHIFT) over the 3-bank tile
                        flat = ps_a.rearrange("p s c -> p (s c)")
                        dump = dpool.tile(
                            [PB, ACT_COLS], BF16, name="dump", tag="dump",
                        )
                        nc.scalar.activation(
                            out=dump,
                            in_=flat,
                            func=mybir.ActivationFunctionType.Exp,
                            bias=negs,
                            scale=1.0,
                            accum_out=pSa[:, bb, g : g + 1],
                        )
                        # VectorE: Schraudolph exp over the 1-bank tile,
                        # summed via the int32->fp32 bitcast
                        idump = ipool.tile(
                            [PB, CHUNK], I32, name="idump", tag="idump",
                        )
                        nc.vector.tensor_scalar(
                            out=idump,
                            in0=ps_v,
                            scalar1=SCH_A,
                            scalar2=SCH_BEFF,
                            op0=mybir.AluOpType.mult,
                            op1=mybir.AluOpType.add,
                        )
                        nc.vector.tensor_reduce(
                            pSv[:, bb, g : g + 1],
                            idump.bitcast(F32),
                            axis=mybir.AxisListType.X,
                            op=mybir.AluOpType.add,
                        )
                    else:
                        # tail tile: one bank, entirely VectorE (idle at the
                        # end while ScalarE drains its last full-tile EXP)
                        ps_v = pool_v.tile(
                            [PB, CHUNK], F32, name="ps_vt", tag="psv",
                        )
                        for k2 in range(KSUB // 2):
                            nc.tensor.matmul(
                                out=ps_v[:, 0:TAILC].unsqueeze(1),
                                lhsT=xap(k2, bb),
                                rhs=wap(g, k2, 0, TAILC),
                                start=(k2 == 0),
                                stop=(k2 == KSUB // 2 - 1),
                                perf_mode=mybir.MatmulPerfMode.DoubleRow,
                            )
                        idump = ipool.tile(
                            [PB, TAILC], I32, name="idumpt", tag="idump",
                            padded_shape=[PB, CHUNK],
                        )
                        nc.vector.tensor_scalar(
                            out=idump,
                            in0=ps_v[:, 0:TAILC],
                            scalar1=SCH_A,
                            scalar2=SCH_BEFF,
                            op0=mybir.AluOpType.mult,
                            op1=mybir.AluOpType.add,
                        )
                        nc.vector.tensor_reduce(
                            pSv[:, bb, NG_FULL : NG_FULL + 1],
                            idump.bitcast(F32),
                            axis=mybir.AxisListType.X,
                            op=mybir.AluOpType.add,
                        )

            # raw per-engine partials out; host does the final sums (the
            # device reduce chain would sit on the critical path)
            nc.sync.dma_start(out=pSv_ext[:], in_=pSv)
            nc.sync.dma_start(out=pSa_ext[:], in_=pSa)

    nc.finalize()
    return nc


def prepare_inputs(x, weight, label, ncores: int = NCORES):
    """Host-side prep: normalize, G-scale, cast fp8, pack to SBUF layouts.

    Returns (in_maps, lc2) where lc2[p, j] = SHIFT - S*cos(x_b, w_label_b)
    for b = j*128 + p."""
    x = np.asarray(x, dtype=np.float32)
    weight = np.asarray(weight, dtype=np.float32)
    label = np.asarray(label).astype(np.int64)

    xn = x / np.maximum(
        np.sqrt(np.einsum("bd,bd->b", x, x, dtype=np.float64))[:, None], EPS
    ).astype(np.float32)
    wnorm = np.sqrt(np.einsum("cd,cd->c", weight, weight, dtype=np.float64))
    wn = weight / np.maximum(wnorm[:, None], EPS).astype(np.float32)

    # label cosine computed on host in f64 (exact vs fp32 reference)
    wl = wn[label]  # [B, D]
    label_cos = np.einsum("bd,bd->b", xn.astype(np.float64), wl.astype(np.float64))
    lc2 = (SHIFT - S * label_cos).astype(np.float64)  # [B]
    lc2_pj = np.ascontiguousarray(lc2.reshape(BBLK, PB).T)  # [128, BBLK]

    x8 = (G * xn).astype(NP_FP8)          # [B, D]
    w8 = (G * wn).astype(NP_FP8)          # [C, D]
    # xnt[p, ks, b] = x8[b, ks*128 + p]
    xp = x8.reshape(B, KSUB, PB).transpose(2, 1, 0)  # [128, 4, 512]

    in_maps = []
    for i in range(ncores):
        wp = np.zeros((CS_PAD, D), dtype=NP_FP8)
        wp[:CS] = w8[i * CS : (i + 1) * CS]
        # group g block: [p, ks, col] = wp[g*2048 + col, ks*128 + p]
        main = (
            wp[: NG_FULL * GCOLS]
            .reshape(NG_FULL, GCOLS, KSUB, PB)
            .transpose(0, 3, 2, 1)  # [6, 128, 4, 2048]
        )
        tail = wp[NG_FULL * GCOLS :].reshape(TAILC, KSUB, PB).transpose(2, 1, 0)
        wx0 = np.concatenate([xp, main[0]], axis=2)          # [128, 4, 2560]
        wmid = main[1:5].reshape(4 * PB, KSUB, GCOLS)
        wrest = np.concatenate([main[5], tail], axis=2)      # [128, 4, 2304]
        in_maps.append(
            {
                "wx0": np.ascontiguousarray(wx0),
                "wmid": np.ascontiguousarray(wmid),
                "wrest": np.ascontiguousarray(wrest),
            }
        )
    return in_maps, lc2_pj


_NC_CACHE = {}


def _get_nc():
    if "nc" not in _NC_CACHE:
        _NC_CACHE["nc"] = build_nc()
    return _NC_CACHE["nc"]


def _install_ntff_hook():
    """The agent image's antenv lacks axon_hooks; shim it so trace=True can
    capture NTFF profiles via the ctypes hook in trn_agent_boot."""
    import sys
    import types

    try:
        from antenv.axon_hooks import get_axon_ntff_profile_hook  # noqa: F401
        return
    except ImportError:
        pass
    mod = types.ModuleType("antenv.axon_hooks")
    _state = {"hook": None}
    mod.set_axon_ntff_profile_hook = lambda h: _state.__setitem__("hook", h)
    mod.get_axon_ntff_profile_hook = lambda: _state["hook"]
    sys.modules["antenv.axon_hooks"] = mod
    import antenv

    antenv.axon_hooks = mod
    from trn_agent_boot.trn_boot import _ntff_profile_via_ctypes

    mod.set_axon_ntff_profile_hook(
        _ntff_profile_via_ctypes("/opt/axon/libaxon_pjrt.so")
    )
    # keep trace artifacts local (no external upload from this sandbox)
    import concourse.bass_utils as bu

    bu.upload_artifacts = lambda tmpdir: tmpdir


def finish_loss(results, lc2_pj):
    """Host epilogue: sum the 8 per-core partials, remove the exact
    zero-pad contribution, log, add label term, mean."""
    Z = np.zeros((PB, BBLK), dtype=np.float64)
    for r in results:
        a = r["pSa"].astype(np.float64)  # [128, 4, 7]
        v = r["pSv"].astype(np.float64)
        Z += a[:, :, :NG_FULL].sum(axis=2) + v.sum(axis=2)
    # pads: tail-tile cols 212..255 are zero logits on the VectorE
    # Schraudolph path -> each contributes the bit-deterministic image of
    # rint(fp32(SCH_BEFF)) reinterpreted as fp32
    n_pad = CS_PAD - CS                      # 44
    pad_img = np.int32(np.rint(np.float32(0.0) * np.float32(SCH_A)
                               + np.float32(SCH_BEFF)))
    pad_val = float(np.frombuffer(pad_img.tobytes(), dtype=np.float32)[0])
    Z -= NCORES * n_pad * pad_val
    loss = float((np.log(Z) + lc2_pj).mean())
    return np.float32(loss)


def run(x, weight, label, trace=False):
    """Returns (loss_scalar, BassKernelResults)."""
    if trace:
        _install_ntff_hook()
    nc = _get_nc()
    in_maps, lc2_pj = prepare_inputs(x, weight, label)
    res = run_bass_kernel_spmd(
        nc, in_maps, core_ids=list(range(NCORES)), trace=trace
    )
    loss = finish_loss(res.results, lc2_pj)
    return loss, res


def kernel(x, weight, label, batch=None, **_ignored):
    loss, _ = run(x, weight, label, trace=False)
    return np.asarray(loss, dtype=np.float32)


# revision 10
# speedup vs baseline: 1.2060x; 1.0101x over previous
"""ArcFace loss kernel for 8 TRN2 NeuronCores (vocab/tensor-parallel).

reference:
    xn = normalize(x)               # [B, D]
    wn = normalize(weight)          # [C, D]
    logits = 64 * xn @ wn.T         # [B, C]
    loss = mean(CE(logits, label))

Strategy: shard classes C=100000 over 8 cores (12500 each, zero-padded to
12544 = 24*512 + 256). Host prepares normalized fp8(e4m3) operands scaled
by G=8 (so device cosines are 64*cos), pre-packed in the exact SBUF tile
layout so every weight-group DMA is 8KB-contiguous per partition. Each core
computes its logit shard with TensorE fp8 DoubleRow matmuls (K=256/op) into
fp32 PSUM.

The sum-of-exp over each PSUM tile is split across two engines so the PE
stream (8 x 216ns matmuls = ~1.73us/tile) is the only bottleneck:
  - ScalarE: one fused exp(l - SHIFT) activation over a dedicated 3-bank
    PSUM tile (1536 cols) with row-accumulate (~1.54us + 182ns drain).
  - VectorE: Schraudolph exp on a dedicated 1-bank PSUM tile (512 cols):
    i32 = rint(A*l + Beff) via one tensor_scalar (fp32 mul-add, HW
    verified round-to-nearest), whose int32 bits ARE the fp32 exp image;
    a tensor_reduce over the bitcast sums it (~1.4us total). The
    mean-centering constant absorbs the (1+f)2^-f mantissa sawtooth
    (E[g]=1.040684); residual loss error ~1e-4 relative.
  The two consumers read DISJOINT PSUM tiles (separate pools): Tile's
  access tracker serializes cross-engine readers of one tile, which
  otherwise stalls the PE every other tile and trips HAM re-throttling.

DMA: every HWDGE trigger costs ~650ns of sequencer time, serially, so
transfers are packed to minimize trigger count ahead of the critical
path: x and w0 ride one dram tensor (wx0, k-halved so the first matmuls
start ~1.2us earlier), w5 and the tail share another. All on the sync
ring in exact consumption order; one ring already sprays all 16 SDMA
engines at ~420GB/s aggregate.

Tail tiles (256 cols) go entirely to VectorE (ScalarE runs near zero
slack) and are interleaved mid-stream where they smooth the consumer
pipelines, with one kept last so the ending is short; the 44 zero-pad
columns contribute a bit-deterministic Schraudolph image of exp(-SHIFT)
that the host subtracts exactly. Raw per-engine accumulator tiles
[128, 4, 7] are DMA'd out directly (no device reduce on the critical
path); the host sums the 8 cores and finishes
loss = mean(log Z + SHIFT - 64*cos_label) with host-exact label cosines.
"""

import math
import numpy as np

import concourse.mybir as mybir
import concourse.tile as tile
from concourse import bacc
from concourse.bass_utils import run_bass_kernel_spmd

# Problem constants (hardcoded per harness contract).
B = 512
D = 512
C = 100000
S = 64.0
SHIFT = 20.0  # logsumexp shift; keeps Z ~1e-2
EPS = 1e-12
G = 8.0      # fp8 pre-scale on both operands: device cos' = G^2 * cos
NCORES = 8
CS = C // NCORES        # true classes per core = 12500
CHUNK = 512             # matmul moving free dim = one full PSUM bank
TAILC = 256             # tail chunk width (212 real + 44 pad cols)
CS_PAD = 24 * CHUNK + TAILC  # padded classes per core = 12544
ASUB = 3                # psum banks per tile consumed by ScalarE
GROUP = 4               # psum banks (512-col chunks) per full tile
NG_FULL = 6             # full groups of 4 chunks; + 1 tail group of 1 chunk
GCOLS = GROUP * CHUNK   # 2048 logit columns per full group
PB = 128                # partitions
KSUB = D // PB          # 4 contraction subtiles of 128
BBLK = B // PB          # 4 batch blocks
NG = NG_FULL + 1        # total groups per core
N_WARM = 14             # PE warm-up matmuls issued while the first DMAs land

ACT_COLS = ASUB * CHUNK       # 1536 cols on ScalarE
XW = B + GCOLS                # wx0 packed width (x | w0)
WR = GCOLS + TAILC            # wrest packed width (w5 | wtail)

# Schraudolph exp-by-bitcast constants (fp32): exp(x) ~= bitcast_f32(
#   rint(A*x + B0)), mean-centered over the mantissa sawtooth.
SCH_A = float(2.0**23 / math.log(2.0))                   # 12102203.1616
SCH_EG = 1.040684490502804                               # E[(1+f)2^-f]
SCH_B0 = (127.0 - math.log(SCH_EG) / math.log(2.0)) * 2.0**23
SCH_BEFF = SCH_B0 - SCH_A * SHIFT                        # folds the shift

F32 = mybir.dt.float32
I32 = mybir.dt.int32
BF16 = mybir.dt.bfloat16
FP8 = mybir.dt.float8e4
NP_FP8 = mybir.dt.np(FP8)


def build_nc(ncores: int = NCORES):
    """Build the SPMD Bass graph."""
    nc = bacc.Bacc(
        "TRN2",
        target_bir_lowering=False,
        debug=False,
        num_devices=ncores,
    )

    # host-packed operands: per-partition-contiguous SBUF layouts
    wx0_ext = nc.dram_tensor("wx0", [PB, KSUB, XW], FP8, kind="ExternalInput")
    wmid_ext = nc.dram_tensor(
        "wmid", [4 * PB, KSUB, GCOLS], FP8, kind="ExternalInput"
    )
    wrest_ext = nc.dram_tensor("wrest", [PB, KSUB, WR], FP8, kind="ExternalInput")
    pSa_ext = nc.dram_tensor("pSa", [PB, BBLK, NG], F32, kind="ExternalOutput")
    pSv_ext = nc.dram_tensor("pSv", [PB, BBLK, NG], F32, kind="ExternalOutput")

    with tile.TileContext(nc) as tc:
        with (
            tc.tile_pool(name="const", bufs=1) as cpool,
            tc.tile_pool(name="wpool", bufs=4) as wpool,
            tc.tile_pool(name="dpool", bufs=3) as dpool,
            tc.tile_pool(name="ipool", bufs=3) as ipool,
        ):
            # ALL loads on the sync HWDGE ring, in consumption order (one
            # ring sprays all 16 SDMA engines; FIFO order == arrival order;
            # each trigger costs ~650ns of sequencer time).
            # k-halves land in SEPARATE tiles: Tile tracks writes at tile
            # granularity, so a shared tile would make the first matmul wait
            # for BOTH halves (~2.5us later)
            wx0a = cpool.tile([PB, 2, XW], FP8)
            wx0b = cpool.tile([PB, 2, XW], FP8)
            nc.sync.dma_start(out=wx0a, in_=wx0_ext[:, 0:2, :])
            nc.sync.dma_start(out=wx0b, in_=wx0_ext[:, 2:4, :])
            wmids = []
            for g in range(4):
                wt = wpool.tile([PB, KSUB, GCOLS], FP8, name="wt", tag="w")
                nc.sync.dma_start(out=wt, in_=wmid_ext[g * PB : (g + 1) * PB, :, :])
                wmids.append(wt)
            wrest = cpool.tile([PB, KSUB, WR], FP8)
            nc.sync.dma_start(out=wrest, in_=wrest_ext[:])

            # per-(k2) x and group-0 weight APs live in the wx0 halves;
            # groups 1-4 in wpool, 5 + tail in wrest
            wx0h = [wx0a, wx0b]

            def xap(k2, bb):
                return wx0h[k2][:, :, bb * PB : (bb + 1) * PB]

            def wap(g, k2, lo, hi):
                if g == 0:
                    return wx0h[k2][:, :, B + lo : B + hi]
                if g < 5:
                    return wmids[g - 1][:, 2 * k2 : 2 * k2 + 2, lo:hi]
                if g == 5:
                    return wrest[:, 2 * k2 : 2 * k2 + 2, lo:hi]
                return wrest[:, 2 * k2 : 2 * k2 + 2, GCOLS + lo : GCOLS + hi]

            # warm-up operand first so the PE can start immediately
            warm = cpool.tile([PB, 256], BF16)
            nc.vector.memset(warm, 0.0)

            # constants
            negs = cpool.tile([PB, 1], F32)
            nc.vector.memset(negs, -SHIFT)

            # per-row partial sums, one tile per engine (sharing one tile
            # creates a false cross-engine WAW serialization)
            pSa = cpool.tile([PB, BBLK, NG], F32)           # ScalarE accum
            pSv = cpool.tile([PB, BBLK, NG], F32)           # VectorE sums

            # preload the Exp activation table off the critical path
            dumdum = cpool.tile([PB, 1], BF16)
            nc.scalar.activation(
                out=dumdum, in_=negs,
                func=mybir.ActivationFunctionType.Exp, bias=negs, scale=1.0,
            )

            with (
                tc.tile_pool(name="psa", bufs=2, space="PSUM") as pool_a,
                tc.tile_pool(name="psv", bufs=2, space="PSUM") as pool_v,
            ):
                # PE warm-up: dependency-free matmuls so the HAM clock gate
                # is released by the time the first weight tiles arrive.
                ones_bf = nc.const_aps.aps[(BF16, 1.0)]
                warm_ps = pool_a.tile(
                    [PB, ASUB, CHUNK], F32, name="warm_ps", tag="psa",
                )
                for _ in range(N_WARM):
                    nc.tensor.matmul(
                        out=warm_ps[0:1, 0, :256], lhsT=ones_bf, rhs=warm,
                        start=True, stop=True,
                    )

                # full groups in DMA-arrival order; tail tiles slotted where
                # VectorE has slack, one kept last so the ending is short
                tiles = [(g, bb) for g in range(2) for bb in range(BBLK)]
                tiles += [(2, 0), (NG - 1, 0), (2, 1), (2, 2), (2, 3)]
                tiles += [(3, 0), (NG - 1, 1), (3, 1), (3, 2), (3, 3)]
                tiles += [(4, 0), (NG - 1, 2), (4, 1), (4, 2), (4, 3)]
                tiles += [(5, bb) for bb in range(BBLK)]
                tiles += [(NG - 1, 3)]
                for g, bb in tiles:
                    if g < NG_FULL:
                        # ScalarE's 3 banks + VectorE's 1 bank, disjoint
                        # PSUM tiles so the consumers never serialize
                        ps_a = pool_a.tile(
                            [PB, ASUB, CHUNK], F32, name="ps_a", tag="psa",
                        )
                        ps_v = pool_v.tile(
                            [PB, CHUNK], F32, name="ps_v", tag="psv",
                        )
                        for k2 in range(KSUB // 2):
                            for sub in range(GROUP):
                                out_ap = (
                                    ps_a[:, sub : sub + 1, :]
                                    if sub < ASUB
                                    else ps_v[:, 0:CHUNK].unsqueeze(1)
                                )
                                nc.tensor.matmul(
                                    out=out_ap,
                                    lhsT=xap(k2, bb),
                                    rhs=wap(
                                        g, k2, sub * CHUNK, (sub + 1) * CHUNK
                                    ),
                                    start=(k2 == 0),
                                    stop=(k2 == KSUB // 2 - 1),
                                    perf_mode=mybir.MatmulPerfMode.DoubleRow,
                                )
                        # ScalarE: exp(l - SHIFT) over the 3-bank tile
                        flat = ps_a.rearrange("p s c -> p (s c)")
                        dump = dpool.tile(
                            [PB, ACT_COLS], BF16, name="dump", tag="dump",
                        )
                        nc.scalar.activation(
                            out=dump,
                            in_=flat,
                            func=mybir.ActivationFunctionType.Exp,
                            bias=negs,
                            scale=1.0,
                            accum_out=pSa[:, bb, g : g + 1],
                        )
                        # VectorE: Schraudolph exp over the 1-bank tile,
                        # summed via the int32->fp32 bitcast
                        idump = ipool.tile(
                            [PB, CHUNK], I32, name="idump", tag="idump",
                        )
                        nc.vector.tensor_scalar(
                            out=idump,
                            in0=ps_v,
                            scalar1=SCH_A,
                            scalar2=SCH_BEFF,
                            op0=mybir.AluOpType.mult,
                            op1=mybir.AluOpType.add,
                        )
                        nc.vector.tensor_reduce(
                            pSv[:, bb, g : g + 1],
                            idump.bitcast(F32),
                            axis=mybir.AxisListType.X,
                            op=mybir.AluOpType.add,
                        )
                    else:
                        # tail tile: one bank, entirely VectorE (idle at the
                        # end while ScalarE drains its last full-tile EXP)
                        ps_v = pool_v.tile(
                            [PB, CHUNK], F32, name="ps_vt", tag="psv",
                        )
                        for k2 in range(KSUB // 2):
                            nc.tensor.matmul(
                                out=ps_v[:, 0:TAILC].unsqueeze(1),
                                lhsT=xap(k2, bb),
                                rhs=wap(g, k2, 0, TAILC),
                                start=(k2 == 0),
                                stop=(k2 == KSUB // 2 - 1),
                                perf_mode=mybir.MatmulPerfMode.DoubleRow,
                            )
                        idump = ipool.tile(
                            [PB, TAILC], I32, name="idumpt", tag="idump",
                            padded_shape=[PB, CHUNK],
                        )
                        nc.vector.tensor_scalar(
                            out=idump,
                            in0=ps_v[:, 0:TAILC],
                            scalar1=SCH_A,
                            scalar2=SCH_BEFF,
                            op0=mybir.AluOpType.mult,
                            op1=mybir.AluOpType.add,
                        )
                        nc.vector.tensor_reduce(
                            pSv[:, bb, NG_FULL : NG_FULL + 1],
                            idump.bitcast(F32),
                            axis=mybir.AxisListType.X,
                            op=mybir.AluOpType.add,
                        )

            # raw per-engine partials out; host does the final sums (the
            # device reduce chain would sit on the critical path)
            nc.sync.dma_start(out=pSv_ext[:], in_=pSv)
            nc.sync.dma_start(out=pSa_ext[:], in_=pSa)

    nc.finalize()
    return nc


def prepare_inputs(x, weight, label, ncores: int = NCORES):
    """Host-side prep: normalize, G-scale, cast fp8, pack to SBUF layouts.

    Returns (in_maps, lc2) where lc2[p, j] = SHIFT - S*cos(x_b, w_label_b)
    for b = j*128 + p."""
    x = np.asarray(x, dtype=np.float32)
    weight = np.asarray(weight, dtype=np.float32)
    label = np.asarray(label).astype(np.int64)

    xn = x / np.maximum(
        np.sqrt(np.einsum("bd,bd->b", x, x, dtype=np.float64))[:, None], EPS
    ).astype(np.float32)
    wnorm = np.sqrt(np.einsum("cd,cd->c", weight, weight, dtype=np.float64))
    wn = weight / np.maximum(wnorm[:, None], EPS).astype(np.float32)

    # label cosine computed on host in f64 (exact vs fp32 reference)
    wl = wn[label]  # [B, D]
    label_cos = np.einsum("bd,bd->b", xn.astype(np.float64), wl.astype(np.float64))
    lc2 = (SHIFT - S * label_cos).astype(np.float64)  # [B]
    lc2_pj = np.ascontiguousarray(lc2.reshape(BBLK, PB).T)  # [128, BBLK]

    x8 = (G * xn).astype(NP_FP8)          # [B, D]
    w8 = (G * wn).astype(NP_FP8)          # [C, D]
    # xnt[p, ks, b] = x8[b, ks*128 + p]
    xp = x8.reshape(B, KSUB, PB).transpose(2, 1, 0)  # [128, 4, 512]

    in_maps = []
    for i in range(ncores):
        wp = np.zeros((CS_PAD, D), dtype=NP_FP8)
        wp[:CS] = w8[i * CS : (i + 1) * CS]
        # group g block: [p, ks, col] = wp[g*2048 + col, ks*128 + p]
        main = (
            wp[: NG_FULL * GCOLS]
            .reshape(NG_FULL, GCOLS, KSUB, PB)
            .transpose(0, 3, 2, 1)  # [6, 128, 4, 2048]
        )
        tail = wp[NG_FULL * GCOLS :].reshape(TAILC, KSUB, PB).transpose(2, 1, 0)
        wx0 = np.concatenate([xp, main[0]], axis=2)          # [128, 4, 2560]
        wmid = main[1:5].reshape(4 * PB, KSUB, GCOLS)
        wrest = np.concatenate([main[5], tail], axis=2)      # [128, 4, 2304]
        in_maps.append(
            {
                "wx0": np.ascontiguousarray(wx0),
                "wmid": np.ascontiguousarray(wmid),
                "wrest": np.ascontiguousarray(wrest),
            }
        )
    return in_maps, lc2_pj


_NC_CACHE = {}


def _get_nc():
    if "nc" not in _NC_CACHE:
        _NC_CACHE["nc"] = build_nc()
    return _NC_CACHE["nc"]


def _install_ntff_hook():
    """The agent image's antenv lacks axon_hooks; shim it so trace=True can
    capture NTFF profiles via the ctypes hook in trn_agent_boot."""
    import sys
    import types

    try:
        from antenv.axon_hooks import get_axon_ntff_profile_hook  # noqa: F401
        return
    except ImportError:
        pass
    mod = types.ModuleType("antenv.axon_hooks")
    _state = {"hook": None}
    mod.set_axon_ntff_profile_hook = lambda h: _state.__setitem__("hook", h)
    mod.get_axon_ntff_profile_hook = lambda: _state["hook"]
    sys.modules["antenv.axon_hooks"] = mod
    import antenv

    antenv.axon_hooks = mod
    from trn_agent_boot.trn_boot import _ntff_profile_via_ctypes

    mod.set_axon_ntff_profile_hook(
        _ntff_profile_via_ctypes("/opt/axon/libaxon_pjrt.so")
    )
    # keep trace artifacts local (no external upload from this sandbox)
    import concourse.bass_utils as bu

    bu.upload_artifacts = lambda tmpdir: tmpdir


def finish_loss(results, lc2_pj):
    """Host epilogue: sum the 8 per-core partials, remove the exact
    zero-pad contribution, log, add label term, mean."""
    Z = np.zeros((PB, BBLK), dtype=np.float64)
    for r in results:
        a = r["pSa"].astype(np.float64)  # [128, 4, 7]
        v = r["pSv"].astype(np.float64)
        Z += a[:, :, :NG_FULL].sum(axis=2) + v.sum(axis=2)
    # pads: tail-tile cols 212..255 are zero logits on the VectorE
    # Schraudolph path -> each contributes the bit-deterministic image of
    # rint(fp32(SCH_BEFF)) reinterpreted as fp32
    n_pad = CS_PAD - CS                      # 44
    pad_img = np.int32(np.rint(np.float32(0.0) * np.float32(SCH_A)
                               + np.float32(SCH_BEFF)))
    pad_val = float(np.frombuffer(pad_img.tobytes(), dtype=np.float32)[0])
    Z -= NCORES * n_pad * pad_val
    loss = float((np.log(Z) + lc2_pj).mean())
    return np.float32(loss)


def run(x, weight, label, trace=False):
    """Returns (loss_scalar, BassKernelResults)."""
    if trace:
        _install_ntff_hook()
    nc = _get_nc()
    in_maps, lc2_pj = prepare_inputs(x, weight, label)
    res = run_bass_kernel_spmd(
        nc, in_maps, core_ids=list(range(NCORES)), trace=trace
    )
    loss = finish_loss(res.results, lc2_pj)
    return loss, res


def kernel(x, weight, label, batch=None, **_ignored):
    loss, _ = run(x, weight, label, trace=False)
    return np.asarray(loss, dtype=np.float32)
